# revision 20
# baseline (speedup 1.0000x reference)
"""KalmanNetNN single-step kernel for 8x TRN2 NeuronCores (Bass/Tile).

Data-parallel: batch 65536 split across 8 cores (8192 rows each), 16 tiles
of 512 batch rows (batch on free dim, features on partitions).

v2: software-pipelined 6-stage emission across tiles so every engine FIFO
holds ready work from adjacent tiles (the v1 kernel ran the GRU chain
nearly serially: PE 46%, DVE 30%, ACT 29% busy).  Matmul merges: in_Sigma
via hq written over FC5-out in the x56 tile (one K=104 matmul), hsig|hs in
one hss tile (fc2a 8->4 matmuls), pnt+bpt share one PSUM bank, fco+dyrp
share one bank (prd reads both PSUM halves in one STT), m1y/prior drop the
lo-part matmuls.  Ln/Exp normalization in 2 groups of 8 tiles to halve the
phase barrier.  Elementwise spread over DVE/ACT/Pool(gpsimd).

Layout rules respected: SBUF rhs/operand partition bases at 0/32/64/96;
SBUF-SBUF vector ops share a base (PSUM operands exempt); accumulating
matmul groups all start at K row 0 (tile_position 0).
  A1 [128,BF] = [h_Q 0:64 | h_Sigma 64:128]
  A2 [128,BF] = [h_S 0:64 | xpp 96:104 | xprp 104:112 | yp 112:120]
  SM [128,BF] = [xp_hi 0:8 | xp_lo 8:16 | y 64:72 | xp 96:104 xp 104:112 y 112:120]
  d  [128,BF] = {dy 0:8, fed 32:40, fud 40:48, od 48:56}, rest stays 0
  x56: FC5-out 0:40 (consumed by Q) then hq overwrites 0:64; FC6-out 64:104
  hss: hs 0:64 | hsig 64:128
"""

import sys
import numpy as np
import ml_dtypes

sys.path.insert(0, "/opt/trn_rl_repo")

B_FULL = 65536
NCORES = 8
BC = B_FULL // NCORES      # rows per core
BF = 512                   # batch tile (free dim)
BF16_NP = ml_dtypes.bfloat16

_cached = {}


def _bf16(x):
    return np.asarray(x, dtype=np.float32).astype(BF16_NP)


class _WImg:
    """Host-side SBUF weight image: [128, ncols], 16-element col alignment."""

    def __init__(self, np_dtype):
        self.np_dtype = np_dtype
        self.cols = 0
        self.blocks = {}   # name -> (row0, nrows, col0, ncols)
        self.data = []

    def place(self, name, row0, arr):
        arr = np.asarray(arr, dtype=self.np_dtype)
        k, m = arr.shape
        col0 = (self.cols + 15) // 16 * 16
        self.cols = col0 + m
        self.blocks[name] = (row0, k, col0, m)
        self.data.append((row0, col0, arr))
        return name

    def image(self):
        ncols = (self.cols + 15) // 16 * 16
        img = np.zeros((128, ncols), dtype=self.np_dtype)
        for row0, col0, arr in self.data:
            k, m = arr.shape
            img[row0:row0 + k, col0:col0 + m] = arr
        return img


def _prep_weights(inp):
    f64 = np.float64
    F = np.asarray(inp["F_mat"], f64)
    H = np.asarray(inp["H_mat"], f64)
    HF = H @ F

    def hi(a):
        return _bf16(a).astype(f64)

    wb = _WImg(BF16_NP)
    # hi-part only; [HFhi; HFhi] against [xp_hi; xp_lo] recovers HFhi @ xp
    wb.place("m1y_a", 0, np.concatenate([hi(HF).T, hi(HF).T], axis=0))
    # transposed-output: rhs [16,8] / [64,8]; lhsT = sm / prd column slices
    wb.place("priorT", 0, np.concatenate([hi(F).T, hi(F).T], axis=0))

    # sumsq: ss rows 64:68 of pA <- groups {dy, fed, fud, od} of sq
    m2 = np.zeros((128, 4))
    m2[0:8, 0] = 1.0
    m2[32:40, 1] = 1.0
    m2[40:48, 2] = 1.0
    m2[48:56, 3] = 1.0
    wb.place("mm2", 0, m2)
    # scale replication: rs rows {0:8, 32:40, 40:48, 48:56}
    m3 = np.zeros((4, 128))
    m3[0, 0:8] = 1.0
    m3[1, 32:40] = 1.0
    m3[2, 40:48] = 1.0
    m3[3, 48:56] = 1.0
    wb.place("mm3", 0, m3)

    fc5 = np.asarray(inp["fc5_w"], f64)
    fc6 = np.asarray(inp["fc6_w"], f64)
    fc7 = np.asarray(inp["fc7_w"], f64)   # cols 0:8 od, 8:16 oid
    w56 = np.zeros((128, 128))
    w56[32:40, 0:40] = fc5.T              # FED -> FC5-out rows 0:40
    w56[40:48, 64:104] = fc6.T            # FUD -> FC6-out rows 64:104
    wb.place("f56", 0, w56)
    w7 = np.zeros((128, 128))
    w7[0:8, 0:80] = fc7[:, 8:16].T        # oid part (d rows 0:8)
    w7[48:56, 0:80] = fc7[:, 0:8].T       # od part (d rows 48:56)
    wb.place("f7", 0, w7)

    def padded(rows0, w, h=128):
        out = np.zeros((h, w.shape[1]))
        out[rows0:rows0 + w.shape[0]] = w
        return out

    wQ, wSig, wS = (np.asarray(inp[f"gru{g}_Wih"], f64) for g in ("Q", "Sig", "S"))
    hQ, hSig, hS = (np.asarray(inp[f"gru{g}_Whh"], f64) for g in ("Q", "Sig", "S"))
    # Q: x = FC5-out at x56[0:40]; h = a1[0:64]
    wb.place("Q_rz_x", 0, wQ[0:128].T)
    wb.place("Q_n_x", 0, wQ[128:192].T)
    wb.place("Q_rz_h", 0, hQ[0:128].T)
    wb.place("Q_n_h", 0, hQ[128:192].T)
    # Sig: x = x56[0:104] = [hq 0:64 | FC6 64:104]; h = a1[64:128] full-height
    sx = np.zeros((128, 192))
    sx[0:64] = wSig[:, 0:64].T
    sx[64:104] = wSig[:, 64:104].T
    wb.place("Sig_rz_x", 0, sx[:, 0:128])
    wb.place("Sig_n_x", 0, sx[:, 128:192])
    wb.place("Sig_rz_h", 0, padded(64, hSig[0:128].T))
    wb.place("Sig_n_h", 0, padded(64, hSig[128:192].T))
    # S: x1 = fc1-out [0:64]; x2 = x7[0:80]; h = a2[0:64]
    wb.place("S_rz_x1", 0, wS[0:128, 0:64].T)
    wb.place("S_n_x1", 0, wS[128:192, 0:64].T)
    wb.place("S_rz_x2", 0, wS[0:128, 64:144].T)
    wb.place("S_n_x2", 0, wS[128:192, 64:144].T)
    wb.place("S_rz_h", 0, hS[0:128].T)
    wb.place("S_n_h", 0, hS[128:192].T)

    # fc1: rhs hss full-height, hsig at rows 64:128
    wb.place("fc1", 0, padded(64, np.asarray(inp["fc1_w"], f64).T))
    w1 = np.asarray(inp["fc2_w1"], f64)
    w2 = np.asarray(inp["fc2_w2"], f64)
    for c in range(4):
        a = np.zeros((128, 128))
        a[0:64] = w1[128 * c:128 * (c + 1), 64:128].T     # hs part
        a[64:128] = w1[128 * c:128 * (c + 1), 0:64].T     # hsig part
        wb.place(f"fc2a{c}", 0, a)
        wb.place(f"fc2b{c}", 0, w2[:, 128 * c:128 * (c + 1)].T)
    dyr = np.zeros((8, 64))
    for m in range(64):
        dyr[m % 8, m] = 1.0
    wb.place("dyrep", 0, dyr)
    wb.place("ident64", 0, np.eye(64))
    fin = np.zeros((64, 8))
    for m in range(64):
        fin[m, m // 8] = 1.0
    wb.place("finT", 0, fin)

    wf = _WImg(np.float32)
    for g in ("Q", "Sig", "S"):
        bih = np.asarray(inp[f"gru{g}_bih"], f64)
        bhh = np.asarray(inp[f"gru{g}_bhh"], f64)
        wf.place(f"rzb_{g}", 0, (bih[0:128] + bhh[0:128])[:, None])
        wf.place(f"nb_{g}", 0, bih[128:192][:, None])
        wf.place(f"bhhn_{g}", 0, bhh[128:192][:, None])
    f56b = np.zeros((128, 1))
    f56b[0:40, 0] = np.asarray(inp["fc5_b"], f64)
    f56b[64:104, 0] = np.asarray(inp["fc6_b"], f64)
    wf.place("f56b", 0, f56b)
    f7b = np.zeros((128, 1))
    f7b[0:80, 0] = np.asarray(inp["fc7_b"], f64)
    wf.place("f7b", 0, f7b)
    wf.place("f1b", 0, np.asarray(inp["fc1_b"], f64)[:, None])
    b1 = np.asarray(inp["fc2_b1"], f64)
    for c in range(4):
        wf.place(f"hidb{c}", 0, b1[128 * c:128 * (c + 1)][:, None])
    wf.place("b2", 0, np.asarray(inp["fc2_b2"], f64)[:, None])
    wf.place("eps", 64, np.full((4, 1), 1e-30))
    return wb, wf


def _prep_batch(inp, lo, hi):
    def g(name):
        return np.asarray(inp[name][lo:hi], np.float32)

    n = hi - lo
    hq = g("h_Q"); hsig = g("h_Sigma"); hs = g("h_S")
    y = g("y")[:, :, 0]; yp = g("y_previous")[:, :, 0]
    xp = g("m1x_posterior")[:, :, 0]
    xpp = g("m1x_posterior_previous")[:, :, 0]
    xprp = g("m1x_prior_previous")[:, :, 0]
    xp_hi32 = _bf16(xp).astype(np.float32)
    xp_lo = _bf16(xp - xp_hi32)
    xp_hi = xp_hi32.astype(BF16_NP)

    a1 = np.concatenate([hq, hsig], axis=1).astype(BF16_NP)
    a2 = np.zeros((n, 128), dtype=BF16_NP)
    a2[:, 0:64] = _bf16(hs)
    a2[:, 96:104] = _bf16(xpp)
    a2[:, 104:112] = _bf16(xprp)
    a2[:, 112:120] = _bf16(yp)
    sm = np.zeros((n, 128), dtype=BF16_NP)
    sm[:, 0:8] = xp_hi
    sm[:, 8:16] = xp_lo
    sm[:, 64:72] = _bf16(y)
    sm[:, 96:104] = xp_hi
    sm[:, 104:112] = xp_hi
    sm[:, 112:120] = _bf16(y)
    return a1, a2, sm


def build(bc, wb, wf, repeat=1):
    import concourse.bacc as bacc
    import concourse.mybir as mybir
    import concourse.tile as tile

    BF16 = mybir.dt.bfloat16
    F32 = mybir.dt.float32
    AF = mybir.ActivationFunctionType
    AL = mybir.AluOpType

    nt = bc // BF
    ng = 4                     # tiles per Ln/Exp group
    wbi = wb.image()
    wfi = wf.image()

    nc = bacc.Bacc()
    A1 = nc.dram_tensor("A1", [bc, 128], BF16, kind="ExternalInput")
    A2 = nc.dram_tensor("A2", [bc, 128], BF16, kind="ExternalInput")
    SM = nc.dram_tensor("SM", [bc, 128], BF16, kind="ExternalInput")
    WB = nc.dram_tensor("WB", [128, wbi.shape[1]], BF16, kind="ExternalInput")
    WF = nc.dram_tensor("WF", [128, wfi.shape[1]], F32, kind="ExternalInput")
    OUT = nc.dram_tensor("OUT", [bc, 8, 1], F32, kind="ExternalOutput")

    with tile.TileContext(nc) as tc:
        with (
            tc.tile_pool(name="wpool", bufs=1) as wpool,
            tc.tile_pool(name="inA", bufs=8) as inA,
            tc.tile_pool(name="nrm", bufs=2) as nrm,
            tc.tile_pool(name="sb", bufs=3) as sb,
            tc.tile_pool(name="pA", bufs=2, space="PSUM") as pAp,
            tc.tile_pool(name="ps", bufs=5, space="PSUM") as ps,
            tc.tile_pool(name="psT", bufs=1, space="PSUM") as psT,
        ):
            wbt = wpool.tile([128, wbi.shape[1]], BF16, tag="wbt")
            wft = wpool.tile([128, wfi.shape[1]], F32, tag="wft")
            nc.sync.dma_start(out=wbt[:], in_=WB[:])
            nc.sync.dma_start(out=wft[:], in_=WF[:])

            def W(name):
                r0, k, c0, m = wb.blocks[name]
                return wbt[r0:r0 + k, c0:c0 + m]

            def Bv(name):
                r0, k, c0, m = wf.blocks[name]
                return wft[r0:r0 + k, c0:c0 + 1]

            for _rep in range(repeat):
                S = [{} for _ in range(nt)]
                norm = [{} for _ in range(nt // ng)]

                dlist = []
                for _t in range(nt):
                    d = inA.tile([128, BF], BF16, tag="d", bufs=16)
                    if _rep == 0:
                        nc.gpsimd.memset(d[:], 0.0)
                    dlist.append(d)

                def phaseA(t):
                    st = S[t]
                    b0 = t * BF
                    a1 = inA.tile([128, BF], BF16, tag="a1", bufs=16)
                    a2 = inA.tile([128, BF], BF16, tag="a2", bufs=16)
                    sm = inA.tile([128, BF], BF16, tag="sm", bufs=16)
                    d = dlist[t]
                    nc.sync.dma_start(out=a1[:], in_=A1[b0:b0 + BF, :],
                                      transpose=True)
                    nc.sync.dma_start(out=a2[:], in_=A2[b0:b0 + BF, :],
                                      transpose=True)
                    nc.sync.dma_start(out=sm[:], in_=SM[b0:b0 + BF, :],
                                      transpose=True)
                    st.update(a1=a1, a2=a2, sm=sm, d=d)
                    if t % ng == 0:
                        ssall = nrm.tile([4, ng * BF], BF16, tag="ssall")
                        norm[t // ng]["ssall"] = ssall
                    pA = pAp.tile([128, BF], F32, tag="pA")
                    nc.tensor.matmul(pA[0:8, :], W("m1y_a"), sm[0:16, :])
                    nc.vector.tensor_sub(d[0:8, :], sm[64:72, :], pA[0:8, :])
                    nc.gpsimd.tensor_sub(d[32:56, :], sm[96:120, :],
                                         a2[96:120, :])
                    sq = sb.tile([128, BF], BF16, tag="sq", bufs=2)
                    nc.vector.tensor_mul(sq[0:128, :], d[0:128, :], d[0:128, :])
                    nc.tensor.matmul(pA[64:68, :], W("mm2"), sq[0:128, :])
                    g, toff = divmod(t, ng)
                    nc.scalar.activation(
                        norm[g]["ssall"][0:4, toff * BF:(toff + 1) * BF],
                        pA[64:68, :], AF.Identity, bias=Bv("eps"))

                def lnexp(g):
                    ssall = norm[g]["ssall"]
                    lss = nrm.tile([4, ng * BF], F32, tag="lss", bufs=1)
                    sall = nrm.tile([4, ng * BF], BF16, tag="sall")
                    nc.scalar.activation(lss[0:4, :], ssall[0:4, :], AF.Ln)
                    nc.scalar.activation(sall[0:4, :], lss[0:4, :], AF.Exp,
                                         scale=-0.5)
                    norm[g]["sall"] = sall

                def st0(t):   # rs, nd, f56/f7 matmuls + relus
                    st = S[t]
                    g, toff = divmod(t, ng)
                    rs = ps.tile([128, BF], F32, tag="ps")
                    nc.tensor.matmul(
                        rs[0:128, :], W("mm3"),
                        norm[g]["sall"][0:4, toff * BF:(toff + 1) * BF])
                    nd = sb.tile([128, BF], BF16, tag="nd", bufs=2)
                    nc.vector.tensor_mul(nd[0:128, :], st["d"][0:128, :],
                                         rs[0:128, :])
                    f56 = ps.tile([128, BF], F32, tag="ps")
                    nc.tensor.matmul(f56[0:128, :], W("f56"), nd[0:128, :])
                    x56 = sb.tile([128, BF], BF16, tag="x56")
                    nc.scalar.activation(x56[0:128, :], f56[0:128, :], AF.Relu,
                                         bias=Bv("f56b"))
                    f7 = ps.tile([128, BF], F32, tag="ps")
                    nc.tensor.matmul(f7[0:128, :], W("f7"), nd[0:128, :])
                    x7 = sb.tile([128, BF], BF16, tag="x7", bufs=4)
                    nc.vector.tensor_scalar(x7[0:128, :], f7[0:128, :],
                                            Bv("f7b"), 0.0, op0=AL.add,
                                            op1=AL.max)
                    st.update(x56=x56, x7=x7)

                def gru(g, xrhs, h_mm, h_el, nb, hp_out, tagsuf):
                    """xrhs: list of (lhsT-name, rhs-AP). nb in {0, 64}."""
                    rz = ps.tile([128, BF], F32, tag="ps")
                    for i, (suf, rhs) in enumerate(xrhs):
                        nc.tensor.matmul(rz[0:128, :], W(f"{g}_rz_{suf}"), rhs,
                                         start=(i == 0), stop=False)
                    nc.tensor.matmul(rz[0:128, :], W(f"{g}_rz_h"), h_mm,
                                     start=False, stop=True)
                    rzs = sb.tile([128, BF], BF16, tag=f"rzs{tagsuf}", bufs=2)
                    nc.scalar.activation(rzs[0:128, :], rz[0:128, :],
                                         AF.Sigmoid, bias=Bv(f"rzb_{g}"))
                    nB = ps.tile([128, BF], F32, tag="ps")
                    for i, (suf, rhs) in enumerate(xrhs):
                        nc.tensor.matmul(nB[0:64, :], W(f"{g}_n_{suf}"), rhs,
                                         start=(i == 0), stop=False)
                    nc.tensor.matmul(nB[64:128, :], W(f"{g}_n_h"), h_mm)
                    tt = sb.tile([128, BF], BF16, tag=f"tt{tagsuf}", bufs=2)
                    nc.vector.scalar_tensor_tensor(
                        tt[0:64, :], nB[64:128, :], Bv(f"bhhn_{g}"),
                        rzs[0:64, :], op0=AL.add, op1=AL.mult)
                    # PE adds r*(Whh_n h + bhh_n) into the Wih_n x psum rows;
                    # tanh then reads PSUM directly (no SBUF round-trip)
                    nc.tensor.matmul(nB[0:64, :], W("ident64"), tt[0:64, :],
                                     start=False, stop=True)
                    nt_ = sb.tile([128, BF], BF16, tag=f"nt{tagsuf}", bufs=2)
                    nc.scalar.activation(nt_[nb:nb + 64, :], nB[0:64, :],
                                         AF.Tanh, bias=Bv(f"nb_{g}"))
                    dt = sb.tile([128, BF], BF16, tag=f"dt{tagsuf}", bufs=2)
                    nc.gpsimd.tensor_sub(dt[64:128, :], h_el,
                                         nt_[nb:nb + 64, :])
                    et = sb.tile([128, BF], BF16, tag=f"et{tagsuf}", bufs=2)
                    nc.vector.tensor_mul(et[nb:nb + 64, :], rzs[64:128, :],
                                         dt[64:128, :])
                    nc.vector.tensor_add(hp_out, nt_[nb:nb + 64, :],
                                         et[nb:nb + 64, :])

                def st1(t):   # GRU Q -> hq overwrites x56[0:64]
                    st = S[t]
                    x56, a1 = st["x56"], st["a1"]
                    gru("Q", [("x", x56[0:40, :])], a1[0:64, :], a1[0:64, :],
                        0, x56[0:64, :], "Q")

                def st2(t):   # GRU Sigma -> hss[64:128]
                    st = S[t]
                    hss = sb.tile([128, BF], BF16, tag="hss")
                    st["hss"] = hss
                    if _rep == 0 and t < 3:
                        # fc1 reads rows 0:64 (x zero weights) before GRU S
                        # writes them; boot garbage there can be inf -> NaN
                        nc.gpsimd.memset(hss[0:64, :], 0.0)
                    gru("Sig", [("x", st["x56"][0:128, :])],
                        st["a1"][0:128, :], st["a1"][64:128, :],
                        64, hss[64:128, :], "G")

                def st3(t):   # fc1 + GRU S -> hss[0:64]
                    st = S[t]
                    hss = st["hss"]
                    f1 = ps.tile([128, BF], F32, tag="ps")
                    nc.tensor.matmul(f1[0:64, :], W("fc1"), hss[0:128, :])
                    x1 = sb.tile([128, BF], BF16, tag="x1", bufs=2)
                    nc.scalar.activation(x1[0:64, :], f1[0:64, :], AF.Relu,
                                         bias=Bv("f1b"))
                    gru("S", [("x1", x1[0:64, :]), ("x2", st["x7"][0:80, :])],
                        st["a2"][0:64, :], st["a2"][0:64, :],
                        0, hss[0:64, :], "S")

                def st4(t):   # fc2 + dyrep + prd
                    st = S[t]
                    hss = st["hss"]
                    fcod = ps.tile([128, BF], F32, tag="ps")
                    h2eng = [nc.scalar, nc.vector, nc.scalar, nc.vector]
                    for c in range(4):
                        hp = ps.tile([128, BF], F32, tag="ps")
                        nc.tensor.matmul(hp[0:128, :], W(f"fc2a{c}"),
                                         hss[0:128, :])
                        h2 = sb.tile([128, BF], BF16, tag=f"h2c{c}", bufs=2)
                        if h2eng[c] is nc.scalar:
                            nc.scalar.activation(h2[0:128, :], hp[0:128, :],
                                                 AF.Relu, bias=Bv(f"hidb{c}"))
                        else:
                            h2eng[c].tensor_scalar(
                                h2[0:128, :], hp[0:128, :], Bv(f"hidb{c}"),
                                0.0, op0=AL.add, op1=AL.max)
                        nc.tensor.matmul(fcod[0:64, :], W(f"fc2b{c}"),
                                         h2[0:128, :], start=(c == 0),
                                         stop=(c == 3))
                    nc.tensor.matmul(fcod[64:128, :], W("dyrep"),
                                     st["d"][0:8, :])
                    dys = sb.tile([128, BF], BF16, tag="dys", bufs=2)
                    nc.scalar.activation(dys[0:64, :], fcod[64:128, :],
                                         AF.Copy)
                    prd = sb.tile([128, BF], BF16, tag="prd")
                    nc.vector.scalar_tensor_tensor(
                        prd[0:64, :], fcod[0:64, :], Bv("b2"),
                        dys[0:64, :], op0=AL.add, op1=AL.mult)
                    st["prd"] = prd

                def st5(t):   # transposed prior + K dy: out[b,j] chunks
                    st = S[t]
                    b0 = t * BF
                    prd, sm = st["prd"], st["sm"]
                    fpsT = psT.tile([128, 32], F32, tag="ott")
                    for c in range(4):
                        cs = slice(128 * c, 128 * (c + 1))
                        nc.tensor.matmul(fpsT[0:128, 8 * c:8 * c + 8],
                                         prd[0:64, cs], W("finT"),
                                         start=True, stop=False)
                        nc.tensor.matmul(fpsT[0:128, 8 * c:8 * c + 8],
                                         sm[0:16, cs], W("priorT"),
                                         start=False, stop=True)
                    ob = sb.tile([128, 32], F32, tag="ob", bufs=2)
                    nc.vector.tensor_copy(ob[0:128, :], fpsT[0:128, :])
                    dst = OUT[b0:b0 + BF, :, 0].rearrange(
                        "(c r) f -> r c f", c=4)
                    src_ = ob[0:128, :].rearrange("r (c f) -> r c f", c=4)
                    nc.sync.dma_start(out=dst, in_=src_)

                stages = [st0, st1, st2, st3, st4, st5]
                NS = len(stages)

                for t in range(ng):
                    phaseA(t)
                lnexp(0)
                for w in range(nt + NS - 1):
                    for k in reversed(range(NS)):
                        t = w - k
                        if 0 <= t < nt:
                            stages[k](t)
                    if w < nt - ng:
                        phaseA(ng + w)
                    if (w + 1) % ng == 0 and w + 1 < nt:
                        lnexp((w + 1) // ng)

    nc.compile()
    return nc


def _get_built(bc, inputs):
    key = bc
    if key not in _cached:
        wb, wf = _prep_weights(inputs)
        nc = build(bc, wb, wf)
        _cached[key] = (nc, wb, wf)
    return _cached[key]


def run(inputs, trace=False):
    from concourse.bass_utils import run_bass_kernel_spmd

    nc, _, _ = _get_built(BC, inputs)
    wb, wf = _prep_weights(inputs)
    wbi = wb.image()
    wfi = wf.image()
    in_maps = []
    for c in range(NCORES):
        a1, a2, sm = _prep_batch(inputs, c * BC, (c + 1) * BC)
        in_maps.append({"A1": a1, "A2": a2, "SM": sm, "WB": wbi, "WF": wfi})
    res = run_bass_kernel_spmd(nc, in_maps, core_ids=list(range(NCORES)),
                               trace=trace)
    outs = [res.results[c]["OUT"] for c in range(NCORES)]
    return np.concatenate(outs, axis=0), res


def kernel(**inputs):
    return run(inputs)[0]


# revision 22
# speedup vs baseline: 1.9591x; 1.9591x over previous
"""KalmanNetNN single-step kernel for 8x TRN2 NeuronCores (Bass/Tile).

Data-parallel: batch 65536 split across 8 cores (8192 rows each), 16 tiles
of 512 batch rows (batch on free dim, features on partitions).

v2: software-pipelined 6-stage emission across tiles so every engine FIFO
holds ready work from adjacent tiles (the v1 kernel ran the GRU chain
nearly serially: PE 46%, DVE 30%, ACT 29% busy).  Matmul merges: in_Sigma
via hq written over FC5-out in the x56 tile (one K=104 matmul), hsig|hs in
one hss tile (fc2a 8->4 matmuls), pnt+bpt share one PSUM bank, fco+dyrp
share one bank (prd reads both PSUM halves in one STT), m1y/prior drop the
lo-part matmuls.  Ln/Exp normalization in 2 groups of 8 tiles to halve the
phase barrier.  Elementwise spread over DVE/ACT/Pool(gpsimd).

Layout rules respected: SBUF rhs/operand partition bases at 0/32/64/96;
SBUF-SBUF vector ops share a base (PSUM operands exempt); accumulating
matmul groups all start at K row 0 (tile_position 0).
  A1 [128,BF] = [h_Q 0:64 | h_Sigma 64:128]
  A2 [128,BF] = [h_S 0:64 | xpp 96:104 | xprp 104:112 | yp 112:120]
  SM [128,BF] = [xp_hi 0:8 | xp_lo 8:16 | y 64:72 | xp 96:104 xp 104:112 y 112:120]
  d  [128,BF] = {dy 0:8, fed 32:40, fud 40:48, od 48:56}, rest stays 0
  x56: FC5-out 0:40 (consumed by Q) then hq overwrites 0:64; FC6-out 64:104
  hss: hs 0:64 | hsig 64:128
"""

import sys
import numpy as np
import ml_dtypes

sys.path.insert(0, "/opt/trn_rl_repo")

B_FULL = 65536
NCORES = 8
BC = B_FULL // NCORES      # rows per core
BF = 512                   # batch tile (free dim)
BF16_NP = ml_dtypes.bfloat16

_cached = {}


def _bf16(x):
    return np.asarray(x, dtype=np.float32).astype(BF16_NP)


class _WImg:
    """Host-side SBUF weight image: [128, ncols], 16-element col alignment."""

    def __init__(self, np_dtype):
        self.np_dtype = np_dtype
        self.cols = 0
        self.blocks = {}   # name -> (row0, nrows, col0, ncols)
        self.data = []

    def place(self, name, row0, arr):
        arr = np.asarray(arr, dtype=self.np_dtype)
        k, m = arr.shape
        col0 = (self.cols + 15) // 16 * 16
        self.cols = col0 + m
        self.blocks[name] = (row0, k, col0, m)
        self.data.append((row0, col0, arr))
        return name

    def image(self):
        ncols = (self.cols + 15) // 16 * 16
        img = np.zeros((128, ncols), dtype=self.np_dtype)
        for row0, col0, arr in self.data:
            k, m = arr.shape
            img[row0:row0 + k, col0:col0 + m] = arr
        return img


def _prep_weights(inp):
    f64 = np.float64
    F = np.asarray(inp["F_mat"], f64)
    H = np.asarray(inp["H_mat"], f64)
    HF = H @ F

    def hi(a):
        return _bf16(a).astype(f64)

    wb = _WImg(BF16_NP)
    # hi-part only; [HFhi; HFhi] against [xp_hi; xp_lo] recovers HFhi @ xp
    wb.place("m1y_a", 0, np.concatenate([hi(HF).T, hi(HF).T], axis=0))
    # transposed-output: rhs [16,8] / [64,8]; lhsT = sm / prd column slices
    wb.place("priorT", 0, np.concatenate([hi(F).T, hi(F).T], axis=0))

    # sumsq: ss rows 64:68 of pA <- groups {dy, fed, fud, od} of sq
    m2 = np.zeros((128, 4))
    m2[0:8, 0] = 1.0
    m2[32:40, 1] = 1.0
    m2[40:48, 2] = 1.0
    m2[48:56, 3] = 1.0
    wb.place("mm2", 0, m2)
    # scale replication: rs rows {0:8, 32:40, 40:48, 48:56}
    m3 = np.zeros((4, 128))
    m3[0, 0:8] = 1.0
    m3[1, 32:40] = 1.0
    m3[2, 40:48] = 1.0
    m3[3, 48:56] = 1.0
    wb.place("mm3", 0, m3)

    fc5 = np.asarray(inp["fc5_w"], f64)
    fc6 = np.asarray(inp["fc6_w"], f64)
    fc7 = np.asarray(inp["fc7_w"], f64)   # cols 0:8 od, 8:16 oid
    w56 = np.zeros((128, 128))
    w56[32:40, 0:40] = fc5.T              # FED -> FC5-out rows 0:40
    w56[40:48, 64:104] = fc6.T            # FUD -> FC6-out rows 64:104
    wb.place("f56", 0, w56)
    w7 = np.zeros((128, 128))
    w7[0:8, 0:80] = fc7[:, 8:16].T        # oid part (d rows 0:8)
    w7[48:56, 0:80] = fc7[:, 0:8].T       # od part (d rows 48:56)
    wb.place("f7", 0, w7)

    def padded(rows0, w, h=128):
        out = np.zeros((h, w.shape[1]))
        out[rows0:rows0 + w.shape[0]] = w
        return out

    wQ, wSig, wS = (np.asarray(inp[f"gru{g}_Wih"], f64) for g in ("Q", "Sig", "S"))
    hQ, hSig, hS = (np.asarray(inp[f"gru{g}_Whh"], f64) for g in ("Q", "Sig", "S"))
    # Q: x = FC5-out at x56[0:40]; h = a1[0:64]
    wb.place("Q_rz_x", 0, wQ[0:128].T)
    wb.place("Q_n_x", 0, wQ[128:192].T)
    wb.place("Q_rz_h", 0, hQ[0:128].T)
    wb.place("Q_n_h", 0, hQ[128:192].T)
    # Sig: x = x56[0:104] = [hq 0:64 | FC6 64:104]; h = a1[64:128] full-height
    sx = np.zeros((128, 192))
    sx[0:64] = wSig[:, 0:64].T
    sx[64:104] = wSig[:, 64:104].T
    wb.place("Sig_rz_x", 0, sx[:, 0:128])
    wb.place("Sig_n_x", 0, sx[:, 128:192])
    wb.place("Sig_rz_h", 0, padded(64, hSig[0:128].T))
    wb.place("Sig_n_h", 0, padded(64, hSig[128:192].T))
    # S: x1 = fc1-out [0:64]; x2 = x7[0:80]; h = a2[0:64]
    wb.place("S_rz_x1", 0, wS[0:128, 0:64].T)
    wb.place("S_n_x1", 0, wS[128:192, 0:64].T)
    wb.place("S_rz_x2", 0, wS[0:128, 64:144].T)
    wb.place("S_n_x2", 0, wS[128:192, 64:144].T)
    wb.place("S_rz_h", 0, hS[0:128].T)
    wb.place("S_n_h", 0, hS[128:192].T)

    # fc1: rhs hss full-height, hsig at rows 64:128
    wb.place("fc1", 0, padded(64, np.asarray(inp["fc1_w"], f64).T))
    w1 = np.asarray(inp["fc2_w1"], f64)
    w2 = np.asarray(inp["fc2_w2"], f64)
    for c in range(4):
        a = np.zeros((128, 128))
        a[0:64] = w1[128 * c:128 * (c + 1), 64:128].T     # hs part
        a[64:128] = w1[128 * c:128 * (c + 1), 0:64].T     # hsig part
        wb.place(f"fc2a{c}", 0, a)
        wb.place(f"fc2b{c}", 0, w2[:, 128 * c:128 * (c + 1)].T)
    dyr = np.zeros((8, 64))
    for m in range(64):
        dyr[m % 8, m] = 1.0
    wb.place("dyrep", 0, dyr)
    wb.place("ident64", 0, np.eye(64))
    fin = np.zeros((64, 8))
    for m in range(64):
        fin[m, m // 8] = 1.0
    wb.place("finT", 0, fin)

    wf = _WImg(np.float32)
    for g in ("Q", "Sig", "S"):
        bih = np.asarray(inp[f"gru{g}_bih"], f64)
        bhh = np.asarray(inp[f"gru{g}_bhh"], f64)
        wf.place(f"rzb_{g}", 0, (bih[0:128] + bhh[0:128])[:, None])
        wf.place(f"nb_{g}", 0, bih[128:192][:, None])
        wf.place(f"bhhn_{g}", 0, bhh[128:192][:, None])
    f56b = np.zeros((128, 1))
    f56b[0:40, 0] = np.asarray(inp["fc5_b"], f64)
    f56b[64:104, 0] = np.asarray(inp["fc6_b"], f64)
    wf.place("f56b", 0, f56b)
    f7b = np.zeros((128, 1))
    f7b[0:80, 0] = np.asarray(inp["fc7_b"], f64)
    wf.place("f7b", 0, f7b)
    wf.place("f1b", 0, np.asarray(inp["fc1_b"], f64)[:, None])
    b1 = np.asarray(inp["fc2_b1"], f64)
    for c in range(4):
        wf.place(f"hidb{c}", 0, b1[128 * c:128 * (c + 1)][:, None])
    wf.place("b2", 0, np.asarray(inp["fc2_b2"], f64)[:, None])
    wf.place("eps", 0, np.full((4, 1), 1e-30))
    return wb, wf


def _prep_batch(inp, lo, hi):
    def g(name):
        return np.asarray(inp[name][lo:hi], np.float32)

    n = hi - lo
    hq = g("h_Q"); hsig = g("h_Sigma"); hs = g("h_S")
    y = g("y")[:, :, 0]; yp = g("y_previous")[:, :, 0]
    xp = g("m1x_posterior")[:, :, 0]
    xpp = g("m1x_posterior_previous")[:, :, 0]
    xprp = g("m1x_prior_previous")[:, :, 0]
    xp_hi32 = _bf16(xp).astype(np.float32)
    xp_lo = _bf16(xp - xp_hi32)
    xp_hi = xp_hi32.astype(BF16_NP)

    a1 = np.concatenate([hq, hsig], axis=1).astype(BF16_NP)
    a2 = np.zeros((n, 128), dtype=BF16_NP)
    a2[:, 0:64] = _bf16(hs)
    a2[:, 96:104] = _bf16(xpp)
    a2[:, 104:112] = _bf16(xprp)
    a2[:, 112:120] = _bf16(yp)
    sm = np.zeros((n, 128), dtype=BF16_NP)
    sm[:, 0:8] = xp_hi
    sm[:, 8:16] = xp_lo
    sm[:, 64:72] = _bf16(y)
    sm[:, 96:104] = xp_hi
    sm[:, 104:112] = xp_hi
    sm[:, 112:120] = _bf16(y)
    return a1, a2, sm


def build(bc, wb, wf, repeat=1):
    import concourse.bacc as bacc
    import concourse.mybir as mybir
    import concourse.tile as tile

    BF16 = mybir.dt.bfloat16
    F32 = mybir.dt.float32
    AF = mybir.ActivationFunctionType
    AL = mybir.AluOpType

    nt = bc // BF
    ng = 4                     # tiles per Ln/Exp group
    wbi = wb.image()
    wfi = wf.image()

    nc = bacc.Bacc()
    A1 = nc.dram_tensor("A1", [bc, 128], BF16, kind="ExternalInput")
    A2 = nc.dram_tensor("A2", [bc, 128], BF16, kind="ExternalInput")
    SM = nc.dram_tensor("SM", [bc, 128], BF16, kind="ExternalInput")
    WB = nc.dram_tensor("WB", [128, wbi.shape[1]], BF16, kind="ExternalInput")
    WF = nc.dram_tensor("WF", [128, wfi.shape[1]], F32, kind="ExternalInput")
    OUT = nc.dram_tensor("OUT", [bc, 8, 1], F32, kind="ExternalOutput")

    with tile.TileContext(nc) as tc:
        with (
            tc.tile_pool(name="wpool", bufs=1) as wpool,
            tc.tile_pool(name="inA", bufs=8) as inA,
            tc.tile_pool(name="nrm", bufs=2) as nrm,
            tc.tile_pool(name="sb", bufs=3) as sb,
            tc.tile_pool(name="pA", bufs=2, space="PSUM") as pAp,
            tc.tile_pool(name="ps", bufs=5, space="PSUM") as ps,
            tc.tile_pool(name="psT", bufs=1, space="PSUM") as psT,
        ):
            wbt = wpool.tile([128, wbi.shape[1]], BF16, tag="wbt")
            wft = wpool.tile([128, wfi.shape[1]], F32, tag="wft")
            nc.sync.dma_start(out=wbt[:], in_=WB[:])
            nc.sync.dma_start(out=wft[:], in_=WF[:])

            def W(name):
                r0, k, c0, m = wb.blocks[name]
                return wbt[r0:r0 + k, c0:c0 + m]

            def Bv(name):
                r0, k, c0, m = wf.blocks[name]
                return wft[r0:r0 + k, c0:c0 + 1]

            for _rep in range(repeat):
                S = [{} for _ in range(nt)]
                norm = [{} for _ in range(nt // ng)]

                dlist = []
                for _t in range(nt):
                    d = inA.tile([128, BF], BF16, tag="d", bufs=16)
                    if _rep == 0:
                        nc.gpsimd.memset(d[:], 0.0)
                    dlist.append(d)

                def phaseA(t):
                    st = S[t]
                    b0 = t * BF
                    a1 = inA.tile([128, BF], BF16, tag="a1", bufs=16)
                    a2 = inA.tile([128, BF], BF16, tag="a2", bufs=16)
                    sm = inA.tile([128, BF], BF16, tag="sm", bufs=16)
                    d = dlist[t]
                    nc.sync.dma_start(out=a1[:], in_=A1[b0:b0 + BF, :],
                                      transpose=True)
                    nc.sync.dma_start(out=a2[:], in_=A2[b0:b0 + BF, :],
                                      transpose=True)
                    nc.sync.dma_start(out=sm[:], in_=SM[b0:b0 + BF, :],
                                      transpose=True)
                    st.update(a1=a1, a2=a2, sm=sm, d=d)
                    if t % ng == 0:
                        ssall = nrm.tile([4, ng * BF], F32, tag="ssall")
                        norm[t // ng]["ssall"] = ssall
                    pA = pAp.tile([128, BF], F32, tag="pA")
                    nc.tensor.matmul(pA[0:8, :], W("m1y_a"), sm[0:16, :])
                    nc.vector.tensor_sub(d[0:8, :], sm[64:72, :], pA[0:8, :])
                    nc.gpsimd.tensor_sub(d[32:56, :], sm[96:120, :],
                                         a2[96:120, :])
                    sq = sb.tile([128, BF], BF16, tag="sq", bufs=2)
                    nc.vector.tensor_mul(sq[0:128, :], d[0:128, :], d[0:128, :])
                    nc.tensor.matmul(pA[64:68, :], W("mm2"), sq[0:128, :])
                    g, toff = divmod(t, ng)
                    nc.scalar.activation(
                        norm[g]["ssall"][0:4, toff * BF:(toff + 1) * BF],
                        pA[64:68, :], AF.Copy)

                def lnexp(g):
                    ssall = norm[g]["ssall"]
                    lss = nrm.tile([4, ng * BF], F32, tag="lss", bufs=1)
                    sall = nrm.tile([4, ng * BF], BF16, tag="sall")
                    nc.scalar.activation(lss[0:4, :], ssall[0:4, :], AF.Ln,
                                         bias=Bv("eps"))
                    nc.scalar.activation(sall[0:4, :], lss[0:4, :], AF.Exp,
                                         scale=-0.5)
                    norm[g]["sall"] = sall

                def st0(t):   # rs, nd, f56/f7 matmuls + relus
                    st = S[t]
                    g, toff = divmod(t, ng)
                    rs = ps.tile([128, BF], F32, tag="ps")
                    nc.tensor.matmul(
                        rs[0:128, :], W("mm3"),
                        norm[g]["sall"][0:4, toff * BF:(toff + 1) * BF])
                    nd = sb.tile([128, BF], BF16, tag="nd", bufs=2)
                    nc.vector.tensor_mul(nd[0:128, :], st["d"][0:128, :],
                                         rs[0:128, :])
                    f56 = ps.tile([128, BF], F32, tag="ps")
                    nc.tensor.matmul(f56[0:128, :], W("f56"), nd[0:128, :])
                    x56 = sb.tile([128, BF], BF16, tag="x56")
                    nc.scalar.activation(x56[0:128, :], f56[0:128, :], AF.Relu,
                                         bias=Bv("f56b"))
                    f7 = ps.tile([128, BF], F32, tag="ps")
                    nc.tensor.matmul(f7[0:128, :], W("f7"), nd[0:128, :])
                    x7 = sb.tile([128, BF], BF16, tag="x7", bufs=4)
                    nc.vector.tensor_scalar(x7[0:128, :], f7[0:128, :],
                                            Bv("f7b"), 0.0, op0=AL.add,
                                            op1=AL.max)
                    st.update(x56=x56, x7=x7)

                def gru(g, xrhs, h_mm, h_el, nb, hp_out, tagsuf):
                    """xrhs: list of (lhsT-name, rhs-AP). nb in {0, 64}."""
                    rz = ps.tile([128, BF], F32, tag="ps")
                    for i, (suf, rhs) in enumerate(xrhs):
                        nc.tensor.matmul(rz[0:128, :], W(f"{g}_rz_{suf}"), rhs,
                                         start=(i == 0), stop=False)
                    nc.tensor.matmul(rz[0:128, :], W(f"{g}_rz_h"), h_mm,
                                     start=False, stop=True)
                    rzs = sb.tile([128, BF], BF16, tag=f"rzs{tagsuf}", bufs=2)
                    nc.scalar.activation(rzs[0:128, :], rz[0:128, :],
                                         AF.Sigmoid, bias=Bv(f"rzb_{g}"))
                    nB = ps.tile([128, BF], F32, tag="ps")
                    for i, (suf, rhs) in enumerate(xrhs):
                        nc.tensor.matmul(nB[0:64, :], W(f"{g}_n_{suf}"), rhs,
                                         start=(i == 0), stop=False)
                    nc.tensor.matmul(nB[64:128, :], W(f"{g}_n_h"), h_mm)
                    tt = sb.tile([128, BF], BF16, tag=f"tt{tagsuf}", bufs=2)
                    nc.vector.scalar_tensor_tensor(
                        tt[0:64, :], nB[64:128, :], Bv(f"bhhn_{g}"),
                        rzs[0:64, :], op0=AL.add, op1=AL.mult)
                    # PE adds r*(Whh_n h + bhh_n) into the Wih_n x psum rows;
                    # tanh then reads PSUM directly (no SBUF round-trip)
                    nc.tensor.matmul(nB[0:64, :], W("ident64"), tt[0:64, :],
                                     start=False, stop=True)
                    nt_ = sb.tile([128, BF], BF16, tag=f"nt{tagsuf}", bufs=2)
                    nc.scalar.activation(nt_[nb:nb + 64, :], nB[0:64, :],
                                         AF.Tanh, bias=Bv(f"nb_{g}"))
                    dt = sb.tile([128, BF], BF16, tag=f"dt{tagsuf}", bufs=2)
                    nc.gpsimd.tensor_sub(dt[64:128, :], h_el,
                                         nt_[nb:nb + 64, :])
                    et = sb.tile([128, BF], BF16, tag=f"et{tagsuf}", bufs=2)
                    nc.vector.tensor_mul(et[nb:nb + 64, :], rzs[64:128, :],
                                         dt[64:128, :])
                    nc.vector.tensor_add(hp_out, nt_[nb:nb + 64, :],
                                         et[nb:nb + 64, :])

                def st1(t):   # GRU Q -> hq overwrites x56[0:64]
                    st = S[t]
                    x56, a1 = st["x56"], st["a1"]
                    gru("Q", [("x", x56[0:40, :])], a1[0:64, :], a1[0:64, :],
                        0, x56[0:64, :], "Q")

                def st2(t):   # GRU Sigma -> hss[64:128]
                    st = S[t]
                    hss = sb.tile([128, BF], BF16, tag="hss")
                    st["hss"] = hss
                    if _rep == 0 and t < 3:
                        # fc1 reads rows 0:64 (x zero weights) before GRU S
                        # writes them; boot garbage there can be inf -> NaN
                        nc.gpsimd.memset(hss[0:64, :], 0.0)
                    gru("Sig", [("x", st["x56"][0:128, :])],
                        st["a1"][0:128, :], st["a1"][64:128, :],
                        64, hss[64:128, :], "G")

                def st3(t):   # fc1 + GRU S -> hss[0:64]
                    st = S[t]
                    hss = st["hss"]
                    f1 = ps.tile([128, BF], F32, tag="ps")
                    nc.tensor.matmul(f1[0:64, :], W("fc1"), hss[0:128, :])
                    x1 = sb.tile([128, BF], BF16, tag="x1", bufs=2)
                    nc.scalar.activation(x1[0:64, :], f1[0:64, :], AF.Relu,
                                         bias=Bv("f1b"))
                    gru("S", [("x1", x1[0:64, :]), ("x2", st["x7"][0:80, :])],
                        st["a2"][0:64, :], st["a2"][0:64, :],
                        0, hss[0:64, :], "S")

                def st4(t):   # fc2 + dyrep + prd
                    st = S[t]
                    hss = st["hss"]
                    fcod = ps.tile([128, BF], F32, tag="ps")
                    h2eng = [nc.scalar, nc.vector, nc.scalar, nc.vector]
                    for c in range(4):
                        hp = ps.tile([128, BF], F32, tag="ps")
                        nc.tensor.matmul(hp[0:128, :], W(f"fc2a{c}"),
                                         hss[0:128, :])
                        h2 = sb.tile([128, BF], BF16, tag=f"h2c{c}", bufs=2)
                        if h2eng[c] is nc.scalar:
                            nc.scalar.activation(h2[0:128, :], hp[0:128, :],
                                                 AF.Relu, bias=Bv(f"hidb{c}"))
                        else:
                            h2eng[c].tensor_scalar(
                                h2[0:128, :], hp[0:128, :], Bv(f"hidb{c}"),
                                0.0, op0=AL.add, op1=AL.max)
                        nc.tensor.matmul(fcod[0:64, :], W(f"fc2b{c}"),
                                         h2[0:128, :], start=(c == 0),
                                         stop=(c == 3))
                    nc.tensor.matmul(fcod[64:128, :], W("dyrep"),
                                     st["d"][0:8, :])
                    dys = sb.tile([128, BF], F32, tag="dys", bufs=2)
                    nc.vector.tensor_copy(dys[0:64, :], fcod[64:128, :])
                    prd = sb.tile([128, BF], BF16, tag="prd")
                    nc.vector.scalar_tensor_tensor(
                        prd[0:64, :], fcod[0:64, :], Bv("b2"),
                        dys[0:64, :], op0=AL.add, op1=AL.mult)
                    st["prd"] = prd

                def st5(t):   # transposed prior + K dy: out[b,j] chunks
                    st = S[t]
                    b0 = t * BF
                    prd, sm = st["prd"], st["sm"]
                    fpsT = psT.tile([128, 32], F32, tag="ott")
                    for c in range(4):
                        cs = slice(128 * c, 128 * (c + 1))
                        nc.tensor.matmul(fpsT[0:128, 8 * c:8 * c + 8],
                                         prd[0:64, cs], W("finT"),
                                         start=True, stop=False)
                        nc.tensor.matmul(fpsT[0:128, 8 * c:8 * c + 8],
                                         sm[0:16, cs], W("priorT"),
                                         start=False, stop=True)
                    ob = sb.tile([128, 32], F32, tag="ob", bufs=2)
                    nc.vector.tensor_copy(ob[0:128, :], fpsT[0:128, :])
                    dst = OUT[b0:b0 + BF, :, 0].rearrange(
                        "(c r) f -> r c f", c=4)
                    src_ = ob[0:128, :].rearrange("r (c f) -> r c f", c=4)
                    nc.sync.dma_start(out=dst, in_=src_)

                stages = [st0, st1, st2, st3, st4, st5]
                NS = len(stages)

                for t in range(ng):
                    phaseA(t)
                lnexp(0)
                for w in range(nt + NS - 1):
                    for k in reversed(range(NS)):
                        t = w - k
                        if 0 <= t < nt:
                            stages[k](t)
                    if w < nt - ng:
                        phaseA(ng + w)
                    if (w + 1) % ng == 0 and w + 1 < nt:
                        lnexp((w + 1) // ng)

    nc.compile()
    return nc


def _get_built(bc, inputs):
    key = bc
    if key not in _cached:
        wb, wf = _prep_weights(inputs)
        nc = build(bc, wb, wf)
        _cached[key] = (nc, wb, wf)
    return _cached[key]


def run(inputs, trace=False):
    from concourse.bass_utils import run_bass_kernel_spmd

    nc, _, _ = _get_built(BC, inputs)
    wb, wf = _prep_weights(inputs)
    wbi = wb.image()
    wfi = wf.image()
    in_maps = []
    for c in range(NCORES):
        a1, a2, sm = _prep_batch(inputs, c * BC, (c + 1) * BC)
        in_maps.append({"A1": a1, "A2": a2, "SM": sm, "WB": wbi, "WF": wfi})
    res = run_bass_kernel_spmd(nc, in_maps, core_ids=list(range(NCORES)),
                               trace=trace)
    outs = [res.results[c]["OUT"] for c in range(NCORES)]
    return np.concatenate(outs, axis=0), res


def kernel(**inputs):
    return run(inputs)[0]


# revision 25
# speedup vs baseline: 2.2926x; 1.1702x over previous
"""KalmanNetNN single-step kernel for 8x TRN2 NeuronCores (Bass/Tile).

Data-parallel: batch 65536 split across 8 cores (8192 rows each), 16 tiles
of 512 batch rows (batch on free dim, features on partitions).

v2: software-pipelined 6-stage emission across tiles so every engine FIFO
holds ready work from adjacent tiles (the v1 kernel ran the GRU chain
nearly serially: PE 46%, DVE 30%, ACT 29% busy).  Matmul merges: in_Sigma
via hq written over FC5-out in the x56 tile (one K=104 matmul), hsig|hs in
one hss tile (fc2a 8->4 matmuls), pnt+bpt share one PSUM bank, fco+dyrp
share one bank (prd reads both PSUM halves in one STT), m1y/prior drop the
lo-part matmuls.  Ln/Exp normalization in 2 groups of 8 tiles to halve the
phase barrier.  Elementwise spread over DVE/ACT/Pool(gpsimd).

Layout rules respected: SBUF rhs/operand partition bases at 0/32/64/96;
SBUF-SBUF vector ops share a base (PSUM operands exempt); accumulating
matmul groups all start at K row 0 (tile_position 0).
  A1 [128,BF] = [h_Q 0:64 | h_Sigma 64:128]
  A2 [128,BF] = [h_S 0:64 | xpp 96:104 | xprp 104:112 | yp 112:120]
  SM [128,BF] = [xp_hi 0:8 | xp_lo 8:16 | y 64:72 | xp 96:104 xp 104:112 y 112:120]
  d  [128,BF] = {dy 0:8, fed 32:40, fud 40:48, od 48:56}, rest stays 0
  x56: FC5-out 0:40 (consumed by Q) then hq overwrites 0:64; FC6-out 64:104
  hss: hs 0:64 | hsig 64:128
"""

import sys
import numpy as np
import ml_dtypes

sys.path.insert(0, "/opt/trn_rl_repo")

B_FULL = 65536
NCORES = 8
BC = B_FULL // NCORES      # rows per core
BF = 512                   # batch tile (free dim)
BF16_NP = ml_dtypes.bfloat16

_cached = {}


def _bf16(x):
    return np.asarray(x, dtype=np.float32).astype(BF16_NP)


class _WImg:
    """Host-side SBUF weight image: [128, ncols], 16-element col alignment."""

    def __init__(self, np_dtype):
        self.np_dtype = np_dtype
        self.cols = 0
        self.blocks = {}   # name -> (row0, nrows, col0, ncols)
        self.data = []

    def place(self, name, row0, arr):
        arr = np.asarray(arr, dtype=self.np_dtype)
        k, m = arr.shape
        col0 = (self.cols + 15) // 16 * 16
        self.cols = col0 + m
        self.blocks[name] = (row0, k, col0, m)
        self.data.append((row0, col0, arr))
        return name

    def image(self):
        ncols = (self.cols + 15) // 16 * 16
        img = np.zeros((128, ncols), dtype=self.np_dtype)
        for row0, col0, arr in self.data:
            k, m = arr.shape
            img[row0:row0 + k, col0:col0 + m] = arr
        return img


def _prep_weights(inp):
    f64 = np.float64
    F = np.asarray(inp["F_mat"], f64)
    H = np.asarray(inp["H_mat"], f64)
    HF = H @ F

    def hi(a):
        return _bf16(a).astype(f64)

    wb = _WImg(BF16_NP)
    # hi-part only; [HFhi; HFhi] against [xp_hi; xp_lo] recovers HFhi @ xp
    wb.place("m1y_a", 0, np.concatenate([hi(HF).T, hi(HF).T], axis=0))
    # transposed-output: rhs [16,8] / [64,8]; lhsT = sm / prd column slices
    wb.place("priorT", 0, np.concatenate([hi(F).T, hi(F).T], axis=0))

    # sumsq: ss rows 64:68 of pA <- groups {dy, fed, fud, od} of sq
    m2 = np.zeros((128, 4))
    m2[0:8, 0] = 1.0
    m2[32:40, 1] = 1.0
    m2[40:48, 2] = 1.0
    m2[48:56, 3] = 1.0
    wb.place("mm2", 0, m2)
    # scale replication: rs rows {0:8, 32:40, 40:48, 48:56}
    m3 = np.zeros((4, 128))
    m3[0, 0:8] = 1.0
    m3[1, 32:40] = 1.0
    m3[2, 40:48] = 1.0
    m3[3, 48:56] = 1.0
    wb.place("mm3", 0, m3)

    fc5 = np.asarray(inp["fc5_w"], f64)
    fc6 = np.asarray(inp["fc6_w"], f64)
    fc7 = np.asarray(inp["fc7_w"], f64)   # cols 0:8 od, 8:16 oid
    w56 = np.zeros((128, 128))
    w56[32:40, 0:40] = fc5.T              # FED -> FC5-out rows 0:40
    w56[40:48, 64:104] = fc6.T            # FUD -> FC6-out rows 64:104
    wb.place("f56", 0, w56)
    w7 = np.zeros((128, 128))
    w7[0:8, 0:80] = fc7[:, 8:16].T        # oid part (d rows 0:8)
    w7[48:56, 0:80] = fc7[:, 0:8].T       # od part (d rows 48:56)
    wb.place("f7", 0, w7)

    def padded(rows0, w, h=128):
        out = np.zeros((h, w.shape[1]))
        out[rows0:rows0 + w.shape[0]] = w
        return out

    wQ, wSig, wS = (np.asarray(inp[f"gru{g}_Wih"], f64) for g in ("Q", "Sig", "S"))
    hQ, hSig, hS = (np.asarray(inp[f"gru{g}_Whh"], f64) for g in ("Q", "Sig", "S"))
    # Q: x = FC5-out at x56[0:40]; h = a1[0:64]
    wb.place("Q_rz_x", 0, wQ[0:128].T)
    wb.place("Q_n_x", 0, wQ[128:192].T)
    wb.place("Q_rz_h", 0, hQ[0:128].T)
    wb.place("Q_n_h", 0, hQ[128:192].T)
    # Sig: x = x56[0:104] = [hq 0:64 | FC6 64:104]; h = a1[64:128] full-height
    sx = np.zeros((128, 192))
    sx[0:64] = wSig[:, 0:64].T
    sx[64:104] = wSig[:, 64:104].T
    wb.place("Sig_rz_x", 0, sx[:, 0:128])
    wb.place("Sig_n_x", 0, sx[:, 128:192])
    wb.place("Sig_rz_h", 0, padded(64, hSig[0:128].T))
    wb.place("Sig_n_h", 0, padded(64, hSig[128:192].T))
    # S: x1 = fc1-out [0:64]; x2 = x7[0:80]; h = a2[0:64]
    wb.place("S_rz_x1", 0, wS[0:128, 0:64].T)
    wb.place("S_n_x1", 0, wS[128:192, 0:64].T)
    wb.place("S_rz_x2", 0, wS[0:128, 64:144].T)
    wb.place("S_n_x2", 0, wS[128:192, 64:144].T)
    wb.place("S_rz_h", 0, hS[0:128].T)
    wb.place("S_n_h", 0, hS[128:192].T)

    # fc1: rhs hss full-height, hsig at rows 64:128
    wb.place("fc1", 0, padded(64, np.asarray(inp["fc1_w"], f64).T))
    w1 = np.asarray(inp["fc2_w1"], f64)
    w2 = np.asarray(inp["fc2_w2"], f64)
    for c in range(4):
        a = np.zeros((128, 128))
        a[0:64] = w1[128 * c:128 * (c + 1), 64:128].T     # hs part
        a[64:128] = w1[128 * c:128 * (c + 1), 0:64].T     # hsig part
        wb.place(f"fc2a{c}", 0, a)
        wb.place(f"fc2b{c}", 0, w2[:, 128 * c:128 * (c + 1)].T)
    dyr = np.zeros((8, 64))
    for m in range(64):
        dyr[m % 8, m] = 1.0
    wb.place("dyrep", 0, dyr)
    wb.place("ident64", 0, np.eye(64))
    fin = np.zeros((64, 8))
    for m in range(64):
        fin[m, m // 8] = 1.0
    wb.place("finT", 0, fin)

    wf = _WImg(np.float32)
    for g in ("Q", "Sig", "S"):
        bih = np.asarray(inp[f"gru{g}_bih"], f64)
        bhh = np.asarray(inp[f"gru{g}_bhh"], f64)
        wf.place(f"rzb_{g}", 0, (bih[0:128] + bhh[0:128])[:, None])
        wf.place(f"nb_{g}", 0, bih[128:192][:, None])
        wf.place(f"bhhn_{g}", 0, bhh[128:192][:, None])
    f56b = np.zeros((128, 1))
    f56b[0:40, 0] = np.asarray(inp["fc5_b"], f64)
    f56b[64:104, 0] = np.asarray(inp["fc6_b"], f64)
    wf.place("f56b", 0, f56b)
    f7b = np.zeros((128, 1))
    f7b[0:80, 0] = np.asarray(inp["fc7_b"], f64)
    wf.place("f7b", 0, f7b)
    wf.place("f1b", 0, np.asarray(inp["fc1_b"], f64)[:, None])
    b1 = np.asarray(inp["fc2_b1"], f64)
    for c in range(4):
        wf.place(f"hidb{c}", 0, b1[128 * c:128 * (c + 1)][:, None])
    wf.place("b2", 0, np.asarray(inp["fc2_b2"], f64)[:, None])
    wf.place("eps", 0, np.full((4, 1), 1e-30))
    return wb, wf


def _prep_batch(inp, lo, hi):
    def g(name):
        return np.asarray(inp[name][lo:hi], np.float32)

    n = hi - lo
    hq = g("h_Q"); hsig = g("h_Sigma"); hs = g("h_S")
    y = g("y")[:, :, 0]; yp = g("y_previous")[:, :, 0]
    xp = g("m1x_posterior")[:, :, 0]
    xpp = g("m1x_posterior_previous")[:, :, 0]
    xprp = g("m1x_prior_previous")[:, :, 0]
    xp_hi32 = _bf16(xp).astype(np.float32)
    xp_lo = _bf16(xp - xp_hi32)
    xp_hi = xp_hi32.astype(BF16_NP)

    a1 = np.concatenate([hq, hsig], axis=1).astype(BF16_NP)
    a2 = np.zeros((n, 128), dtype=BF16_NP)
    a2[:, 0:64] = _bf16(hs)
    a2[:, 96:104] = _bf16(xpp)
    a2[:, 104:112] = _bf16(xprp)
    a2[:, 112:120] = _bf16(yp)
    sm = np.zeros((n, 128), dtype=BF16_NP)
    sm[:, 0:8] = xp_hi
    sm[:, 8:16] = xp_lo
    sm[:, 64:72] = _bf16(y)
    sm[:, 96:104] = xp_hi
    sm[:, 104:112] = xp_hi
    sm[:, 112:120] = _bf16(y)
    return a1, a2, sm


def build(bc, wb, wf, repeat=1):
    import concourse.bacc as bacc
    import concourse.mybir as mybir
    import concourse.tile as tile

    BF16 = mybir.dt.bfloat16
    F32 = mybir.dt.float32
    AF = mybir.ActivationFunctionType
    AL = mybir.AluOpType

    nt = bc // BF
    ng = 4                     # tiles per Ln/Exp group
    wbi = wb.image()
    wfi = wf.image()

    nc = bacc.Bacc()
    A1 = nc.dram_tensor("A1", [bc, 128], BF16, kind="ExternalInput")
    A2 = nc.dram_tensor("A2", [bc, 128], BF16, kind="ExternalInput")
    SM = nc.dram_tensor("SM", [bc, 128], BF16, kind="ExternalInput")
    WB = nc.dram_tensor("WB", [128, wbi.shape[1]], BF16, kind="ExternalInput")
    WF = nc.dram_tensor("WF", [128, wfi.shape[1]], F32, kind="ExternalInput")
    OUT = nc.dram_tensor("OUT", [bc, 8, 1], F32, kind="ExternalOutput")

    with tile.TileContext(nc) as tc:
        with (
            tc.tile_pool(name="wpool", bufs=1) as wpool,
            tc.tile_pool(name="inA", bufs=8) as inA,
            tc.tile_pool(name="nrm", bufs=2) as nrm,
            tc.tile_pool(name="sb", bufs=3) as sb,
            tc.tile_pool(name="pA", bufs=2, space="PSUM") as pAp,
            tc.tile_pool(name="ps", bufs=6, space="PSUM") as ps,
        ):
            wbt = wpool.tile([128, wbi.shape[1]], BF16, tag="wbt")
            wft = wpool.tile([128, wfi.shape[1]], F32, tag="wft")
            nc.sync.dma_start(out=wbt[:], in_=WB[:])
            nc.sync.dma_start(out=wft[:], in_=WF[:])

            def W(name):
                r0, k, c0, m = wb.blocks[name]
                return wbt[r0:r0 + k, c0:c0 + m]

            def Bv(name):
                r0, k, c0, m = wf.blocks[name]
                return wft[r0:r0 + k, c0:c0 + 1]

            for _rep in range(repeat):
                S = [{} for _ in range(nt)]
                norm = [{} for _ in range(nt // ng)]

                dlist = []
                for _t in range(nt):
                    d = inA.tile([128, BF], BF16, tag="d", bufs=16)
                    if _rep == 0:
                        nc.gpsimd.memset(d[:], 0.0)
                    dlist.append(d)

                def phaseA(t):
                    st = S[t]
                    b0 = t * BF
                    a1 = inA.tile([128, BF], BF16, tag="a1", bufs=16)
                    a2 = inA.tile([128, BF], BF16, tag="a2", bufs=16)
                    sm = inA.tile([128, BF], BF16, tag="sm", bufs=16)
                    d = dlist[t]
                    nc.sync.dma_start(out=sm[:], in_=SM[b0:b0 + BF, :],
                                      transpose=True)
                    nc.sync.dma_start(out=a2[:], in_=A2[b0:b0 + BF, :],
                                      transpose=True)
                    nc.sync.dma_start(out=a1[:], in_=A1[b0:b0 + BF, :],
                                      transpose=True)
                    st.update(a1=a1, a2=a2, sm=sm, d=d)
                    if t % ng == 0:
                        ssall = nrm.tile([4, ng * BF], F32, tag="ssall")
                        norm[t // ng]["ssall"] = ssall
                    pA = pAp.tile([128, BF], F32, tag="pA")
                    nc.tensor.matmul(pA[0:8, :], W("m1y_a"), sm[0:16, :])
                    nc.vector.tensor_sub(d[0:8, :], sm[64:72, :], pA[0:8, :])
                    nc.gpsimd.tensor_sub(d[32:56, :], sm[96:120, :],
                                         a2[96:120, :])
                    sq = sb.tile([128, BF], BF16, tag="sq", bufs=2)
                    nc.vector.tensor_mul(sq[0:128, :], d[0:128, :], d[0:128, :])
                    nc.tensor.matmul(pA[64:68, :], W("mm2"), sq[0:128, :])
                    g, toff = divmod(t, ng)
                    nc.scalar.activation(
                        norm[g]["ssall"][0:4, toff * BF:(toff + 1) * BF],
                        pA[64:68, :], AF.Copy)

                def lnexp(g):
                    ssall = norm[g]["ssall"]
                    lss = nrm.tile([4, ng * BF], F32, tag="lss", bufs=1)
                    sall = nrm.tile([4, ng * BF], BF16, tag="sall")
                    nc.scalar.activation(lss[0:4, :], ssall[0:4, :], AF.Ln,
                                         bias=Bv("eps"))
                    nc.scalar.activation(sall[0:4, :], lss[0:4, :], AF.Exp,
                                         scale=-0.5)
                    norm[g]["sall"] = sall

                def st0(t):   # rs, nd, f56/f7 matmuls + relus
                    st = S[t]
                    g, toff = divmod(t, ng)
                    rs = ps.tile([128, BF], F32, tag="ps")
                    nc.tensor.matmul(
                        rs[0:128, :], W("mm3"),
                        norm[g]["sall"][0:4, toff * BF:(toff + 1) * BF])
                    nd = sb.tile([128, BF], BF16, tag="nd", bufs=2)
                    nc.vector.tensor_mul(nd[0:128, :], st["d"][0:128, :],
                                         rs[0:128, :])
                    f56 = ps.tile([128, BF], F32, tag="ps")
                    nc.tensor.matmul(f56[0:128, :], W("f56"), nd[0:128, :])
                    x56 = sb.tile([128, BF], BF16, tag="x56")
                    nc.scalar.activation(x56[0:128, :], f56[0:128, :], AF.Relu,
                                         bias=Bv("f56b"))
                    f7 = ps.tile([128, BF], F32, tag="ps")
                    nc.tensor.matmul(f7[0:128, :], W("f7"), nd[0:128, :])
                    x7 = sb.tile([128, BF], BF16, tag="x7", bufs=4)
                    nc.vector.tensor_scalar(x7[0:128, :], f7[0:128, :],
                                            Bv("f7b"), 0.0, op0=AL.add,
                                            op1=AL.max)
                    st.update(x56=x56, x7=x7)

                def gru(g, xrhs, h_mm, h_el, nb, hp_out, tagsuf):
                    """xrhs: list of (lhsT-name, rhs-AP). nb in {0, 64}."""
                    rz = ps.tile([128, BF], F32, tag="ps")
                    for i, (suf, rhs) in enumerate(xrhs):
                        nc.tensor.matmul(rz[0:128, :], W(f"{g}_rz_{suf}"), rhs,
                                         start=(i == 0), stop=False)
                    nc.tensor.matmul(rz[0:128, :], W(f"{g}_rz_h"), h_mm,
                                     start=False, stop=True)
                    rzs = sb.tile([128, BF], BF16, tag=f"rzs{tagsuf}", bufs=2)
                    nc.scalar.activation(rzs[0:128, :], rz[0:128, :],
                                         AF.Sigmoid, bias=Bv(f"rzb_{g}"))
                    nB = ps.tile([128, BF], F32, tag="ps")
                    for i, (suf, rhs) in enumerate(xrhs):
                        nc.tensor.matmul(nB[0:64, :], W(f"{g}_n_{suf}"), rhs,
                                         start=(i == 0), stop=False)
                    nc.tensor.matmul(nB[64:128, :], W(f"{g}_n_h"), h_mm)
                    tt = sb.tile([128, BF], BF16, tag=f"tt{tagsuf}", bufs=2)
                    nc.vector.scalar_tensor_tensor(
                        tt[0:64, :], nB[64:128, :], Bv(f"bhhn_{g}"),
                        rzs[0:64, :], op0=AL.add, op1=AL.mult)
                    # PE adds r*(Whh_n h + bhh_n) into the Wih_n x psum rows;
                    # tanh then reads PSUM directly (no SBUF round-trip)
                    nc.tensor.matmul(nB[0:64, :], W("ident64"), tt[0:64, :],
                                     start=False, stop=True)
                    nt_ = sb.tile([128, BF], BF16, tag=f"nt{tagsuf}", bufs=2)
                    nc.scalar.activation(nt_[nb:nb + 64, :], nB[0:64, :],
                                         AF.Tanh, bias=Bv(f"nb_{g}"))
                    dt = sb.tile([128, BF], BF16, tag=f"dt{tagsuf}", bufs=2)
                    nc.gpsimd.tensor_sub(dt[64:128, :], h_el,
                                         nt_[nb:nb + 64, :])
                    et = sb.tile([128, BF], BF16, tag=f"et{tagsuf}", bufs=2)
                    nc.vector.tensor_mul(et[nb:nb + 64, :], rzs[64:128, :],
                                         dt[64:128, :])
                    nc.vector.tensor_add(hp_out, nt_[nb:nb + 64, :],
                                         et[nb:nb + 64, :])

                def st1(t):   # GRU Q -> hq overwrites x56[0:64]
                    st = S[t]
                    x56, a1 = st["x56"], st["a1"]
                    gru("Q", [("x", x56[0:40, :])], a1[0:64, :], a1[0:64, :],
                        0, x56[0:64, :], "Q")

                def st2(t):   # GRU Sigma -> hss[64:128]
                    st = S[t]
                    hss = sb.tile([128, BF], BF16, tag="hss")
                    st["hss"] = hss
                    if _rep == 0 and t < 3:
                        # fc1 reads rows 0:64 (x zero weights) before GRU S
                        # writes them; boot garbage there can be inf -> NaN
                        nc.gpsimd.memset(hss[0:64, :], 0.0)
                    gru("Sig", [("x", st["x56"][0:128, :])],
                        st["a1"][0:128, :], st["a1"][64:128, :],
                        64, hss[64:128, :], "G")

                def st3(t):   # fc1 + GRU S -> hss[0:64]
                    st = S[t]
                    hss = st["hss"]
                    f1 = ps.tile([128, BF], F32, tag="ps")
                    nc.tensor.matmul(f1[0:64, :], W("fc1"), hss[0:128, :])
                    x1 = sb.tile([128, BF], BF16, tag="x1", bufs=2)
                    nc.scalar.activation(x1[0:64, :], f1[0:64, :], AF.Relu,
                                         bias=Bv("f1b"))
                    gru("S", [("x1", x1[0:64, :]), ("x2", st["x7"][0:80, :])],
                        st["a2"][0:64, :], st["a2"][0:64, :],
                        0, hss[0:64, :], "S")

                def st4(t):   # fc2 + dyrep + prd
                    st = S[t]
                    hss = st["hss"]
                    fcod = ps.tile([128, BF], F32, tag="ps")
                    h2eng = [nc.scalar, nc.vector, nc.scalar, nc.vector]
                    for c in range(4):
                        hp = ps.tile([128, BF], F32, tag="ps")
                        nc.tensor.matmul(hp[0:128, :], W(f"fc2a{c}"),
                                         hss[0:128, :])
                        h2 = sb.tile([128, BF], BF16, tag=f"h2c{c}", bufs=2)
                        if h2eng[c] is nc.scalar:
                            nc.scalar.activation(h2[0:128, :], hp[0:128, :],
                                                 AF.Relu, bias=Bv(f"hidb{c}"))
                        else:
                            h2eng[c].tensor_scalar(
                                h2[0:128, :], hp[0:128, :], Bv(f"hidb{c}"),
                                0.0, op0=AL.add, op1=AL.max)
                        nc.tensor.matmul(fcod[0:64, :], W(f"fc2b{c}"),
                                         h2[0:128, :], start=(c == 0),
                                         stop=(c == 3))
                    nc.tensor.matmul(fcod[64:128, :], W("dyrep"),
                                     st["d"][0:8, :])
                    dys = sb.tile([128, BF], F32, tag="dys", bufs=2)
                    nc.vector.tensor_copy(dys[0:64, :], fcod[64:128, :])
                    prd = sb.tile([128, BF], BF16, tag="prd")
                    nc.vector.scalar_tensor_tensor(
                        prd[0:64, :], fcod[0:64, :], Bv("b2"),
                        dys[0:64, :], op0=AL.add, op1=AL.mult)
                    st["prd"] = prd

                def st5(t):   # transposed prior + K dy: out[b,j] chunks
                    st = S[t]
                    b0 = t * BF
                    prd, sm = st["prd"], st["sm"]
                    fpsT = ps.tile([128, 32], F32, tag="ps", padded_shape=[128, BF])
                    for c in range(4):
                        cs = slice(128 * c, 128 * (c + 1))
                        nc.tensor.matmul(fpsT[0:128, 8 * c:8 * c + 8],
                                         prd[0:64, cs], W("finT"),
                                         start=True, stop=False)
                        nc.tensor.matmul(fpsT[0:128, 8 * c:8 * c + 8],
                                         sm[0:16, cs], W("priorT"),
                                         start=False, stop=True)
                    ob = sb.tile([128, 32], F32, tag="ob", bufs=2)
                    nc.vector.tensor_copy(ob[0:128, :], fpsT[0:128, :])
                    dst = OUT[b0:b0 + BF, :, 0].rearrange(
                        "(c r) f -> r c f", c=4)
                    src_ = ob[0:128, :].rearrange("r (c f) -> r c f", c=4)
                    nc.sync.dma_start(out=dst, in_=src_)

                stages = [st0, st1, st2, st3, st4, st5]
                NS = len(stages)

                for t in range(ng):
                    phaseA(t)
                lnexp(0)
                for w in range(nt + NS - 1):
                    for k in reversed(range(NS)):
                        t = w - k
                        if 0 <= t < nt:
                            stages[k](t)
                    if w < nt - ng:
                        phaseA(ng + w)
                    if (w + 1) % ng == 0 and w + 1 < nt:
                        lnexp((w + 1) // ng)

    nc.compile()
    return nc


def _get_built(bc, inputs):
    key = bc
    if key not in _cached:
        wb, wf = _prep_weights(inputs)
        nc = build(bc, wb, wf)
        _cached[key] = (nc, wb, wf)
    return _cached[key]


def run(inputs, trace=False):
    from concourse.bass_utils import run_bass_kernel_spmd

    nc, _, _ = _get_built(BC, inputs)
    wb, wf = _prep_weights(inputs)
    wbi = wb.image()
    wfi = wf.image()
    in_maps = []
    for c in range(NCORES):
        a1, a2, sm = _prep_batch(inputs, c * BC, (c + 1) * BC)
        in_maps.append({"A1": a1, "A2": a2, "SM": sm, "WB": wbi, "WF": wfi})
    res = run_bass_kernel_spmd(nc, in_maps, core_ids=list(range(NCORES)),
                               trace=trace)
    outs = [res.results[c]["OUT"] for c in range(NCORES)]
    return np.concatenate(outs, axis=0), res


def kernel(**inputs):
    return run(inputs)[0]
